# revision 38
# baseline (speedup 1.0000x reference)
"""Bahdanau attention Trainium2 kernel (v2).

score(s, h_i) = v . tanh(W_s s + W_h h_i);  softmax over S;  context = w @ enc.

Strategy (per NeuronCore, data-parallel over batch, 8 batches/core):
  - Host pre-marshals enc into TWO bf16 layouts so the device does no
    casts and no SBUF transposes (which were ~230us of DMA + ~160us of
    DVE in v1):
      enc_t[b, blk, p, c, q, s]  (h = c*128+p, s_glob = blk*512+q*128+s)
        -> energy-GEMM moving tiles [128h, (c,q,s)], contiguous 512-run
           per partition per h-chunk
      enc_n[b, blk, p, q, h]     (s_glob = blk*512+q*128+p)
        -> context-GEMM moving tiles [128s, (q,h)]
    Total HBM traffic is unchanged (two bf16 copies = one f32 copy);
    everything rides the gpsimd SWDGE ring (sprays all 16 DMA engines).
  - dec_proj = dec @ W_s is computed on host (0.01% of FLOPs) and fed
    as the transposed ACT bias dpt[a_p, ca, b].
  - energy runs transposed per 512-s block: psum [a=128, s=512], W_h
    chunks (bf16) stationary; tanh folds dpt in as the ACT bias.
  - scoresT is produced directly as columns: per q, scT[s=128, 1] =
    sum_ca et[ca][:, q]^T v[ca] via N=1 matmuls whose 128-col LS hides
    under the 512-cycle energy matmuls they are interleaved with.
  - the log-mask (0 / -30000) columns are added on the idle DVE before
    exp, so masked positions exp to exactly 0; exp on ACT -> e columns
    (bf16 for the context lhsT, f32 for the weight output); no
    max-subtraction (|scores| <~ 20, f32 exp safe).
  - context accumulates into psum rows 0/32/64/96 via 4 PE column
    groups (their N=512 streams overlap on HW, measured ~2.8x); block-k
    context matmuls are issued inside block k+1's energy stream so the
    PE never waits on the tanh->exp chain.
  - batch tail: colsum-replicating f32 matmul (ones^T @ ewgt) -> DVE
    reduce+reciprocal -> ACT scales; the 4 context partial rows are
    summed by a sel matmul from a zeroed bf16 staging tile; the weight
    row is PE-transposed so its HBM write is 16 contiguous descriptors.
"""

import sys
from contextlib import ExitStack

sys.path.insert(0, "/opt/trn_rl_repo")

import numpy as np

import concourse.bass as bass
import concourse.tile as tile
from concourse import mybir
from concourse.masks import make_identity

# ---- walrus workaround: tail drain accepts only 1 sync wait ----------------
from concourse.vector_clock import ScopedClock, VectorClock


def _patched_drain_and_barrier(self, tick_clock, wait_clock):
    gc = tick_clock.global_clock
    procs = [(i, gc[i]) for i in range(len(gc)) if gc[i] > 0]
    for p, t in procs:
        vc = VectorClock()
        vc.require_at_least(p, t)
        nop = self.nc.sync.nop(nofuse=True, hint="tail_wait_split")
        wait_clock.add_sem_waits(nop.ins, ScopedClock({None: vc}))
    self.nc.sync.drain()
    self.nc.all_engine_barrier()
    assert self.sems is not None
    popped = self.nc._tile_sem_poison_stack.pop()
    assert popped is self._sem_poison
    self.nc.clear_and_free_semaphores(list(self.sems.allocated().values()))
    self.nc.all_engine_barrier()


tile.TileContext._drain_and_barrier = _patched_drain_and_barrier


def _spill_excess_waits(nc):
    """This walrus build accepts at most 1 sync wait per instruction (2 for
    EventSemaphore).  Tile's wait assignment can attach several.  Move the
    excess onto same-engine NOPs inserted immediately before the
    instruction — NX sequencers process instructions in order, so the NOP
    stalls the engine exactly like an on-instruction wait would."""
    import bass_rust

    nop_id = [0]
    for fn in nc.m.functions:
        for blk in fn.blocks:
            new_insts = []
            changed = False
            for inst in blk.instructions:
                si = inst.sync_info
                cap = 2 if type(inst).__name__ == "InstEventSemaphore" else 1
                if si is not None and len(si.on_wait) > cap:
                    waits = list(si.on_wait)
                    keep, spill = waits[-cap:], waits[:-cap]
                    for w in spill:
                        nop = mybir.InstNoOp(
                            name=f"I-waitspill-{nop_id[0]}", ins=[], outs=[]
                        )
                        nop_id[0] += 1
                        nop.engine = inst.engine
                        nop.sync_info = bass_rust.SyncInfo(
                            on_wait=[w], on_update=[]
                        )
                        nc.register_instruction(nop, overwrite=True)
                        new_insts.append(nop)
                    inst.sync_info = bass_rust.SyncInfo(
                        on_wait=keep, on_update=list(si.on_update)
                    )
                    changed = True
                new_insts.append(inst)
            if changed:
                blk.instructions = new_insts
    return nc


# ---------------------------------------------------------------------------

N_CORES = 8
B, S, H, A = 64, 2048, 1024, 512
H2 = 2 * H
F32 = mybir.dt.float32
BF16 = mybir.dt.bfloat16
AF = mybir.ActivationFunctionType
MASK_NEG = -30000.0  # exp(score + MASK_NEG) == 0.0 in f32 for |score| <~ 1e3


def build_bass(bloc, s_len, reps=1, mode="full"):
    """One-core program processing bloc batch rows of length s_len.

    reps>1 re-emits the whole compute loop (idempotent writes) so the
    per-iteration device time can be measured as a wall-clock slope."""
    P = 128
    SB = 512  # s-block
    n_blk = s_len // SB
    n_hc = H2 // P  # 16 h-chunks in the projection contraction
    n_ac = A // P  # 4 a-chunks
    n_hh = H2 // 512  # 4 context output slices
    NQ = SB // P  # 4 s-subtiles per block
    NC16 = n_blk * NQ  # 16 e-columns per batch

    nc = bass.Bass("TRN2", target_bir_lowering=False, debug=False)
    enc_t = nc.dram_tensor(
        "enc_t", [bloc, n_blk, P, NQ * n_hc * P], BF16, kind="ExternalInput"
    ).ap()
    enc_n = nc.dram_tensor(
        "enc_n", [bloc, n_blk, P, NQ * H2], BF16, kind="ExternalInput"
    ).ap()
    dpt_in = nc.dram_tensor("dpt", [P, n_ac, bloc], F32, kind="ExternalInput").ap()
    w_h = nc.dram_tensor("w_h", [P, n_hc * A], BF16, kind="ExternalInput").ap()
    v_in = nc.dram_tensor("v_in", [P, n_ac], BF16, kind="ExternalInput").ap()
    mcol = nc.dram_tensor("mcol", [bloc, P, NC16], F32, kind="ExternalInput").ap()
    csel = nc.dram_tensor("csel", [P, 1], BF16, kind="ExternalInput").ap()
    ctx_o = nc.dram_tensor("ctx_o", [bloc, H2], F32, kind="ExternalOutput").ap()
    wgt_o = nc.dram_tensor("wgt_o", [bloc, s_len], F32, kind="ExternalOutput").ap()

    with tile.TileContext(nc) as tc, ExitStack() as ctx:
        consts = ctx.enter_context(tc.tile_pool(name="consts", bufs=1))

        whsb = consts.tile([P, n_hc, A], BF16)
        nc.sync.dma_start(whsb.rearrange("p c a -> p (c a)"), w_h[:, :])
        vsb = consts.tile([P, n_ac], BF16)
        nc.sync.dma_start(vsb[:], v_in[:, :])
        dpt = consts.tile([P, n_ac, bloc], F32)
        nc.sync.dma_start(dpt.rearrange("p c b -> p (c b)"), dpt_in[:, :, :])
        sel_bf = consts.tile([P, 1], BF16)
        nc.sync.dma_start(sel_bf[:], csel[:])
        ones_f32 = consts.tile([P, P], F32)
        nc.gpsimd.memset(ones_f32[:], 1.0)
        ident = consts.tile([P, P], F32)
        make_identity(nc, ident[:])
        # persistent bf16 staging for the 4 context partial rows; zeroed
        # once so the sel matmul's unused partitions contribute exact 0.
        c4 = consts.tile([P, H2], BF16)
        nc.gpsimd.memset(c4[:], 0.0)

        encT_pool = ctx.enter_context(tc.tile_pool(name="encT", bufs=4))
        nat_pool = ctx.enter_context(tc.tile_pool(name="nat", bufs=5))
        et_pool = ctx.enter_context(tc.tile_pool(name="et", bufs=6))
        ec_pool = ctx.enter_context(tc.tile_pool(name="ec", bufs=3))
        row_pool = ctx.enter_context(tc.tile_pool(name="row", bufs=2))
        out_pool = ctx.enter_context(tc.tile_pool(name="outp", bufs=2))
        energy_ps = ctx.enter_context(
            tc.tile_pool(name="energy_ps", bufs=2, space="PSUM")
        )
        sc_ps = ctx.enter_context(tc.tile_pool(name="sc_ps", bufs=2, space="PSUM"))
        ctx_ps = ctx.enter_context(tc.tile_pool(name="ctx_ps", bufs=1, space="PSUM"))

        # context psum lives across the whole kernel; the 4 q-groups write
        # partial rows 0/32/64/96 in separate PE column groups (their N=512
        # streams overlap on HW); each batch's first matmul per group
        # re-initializes its row via start=True.
        cps = ctx_ps.tile([P, H2], F32, tag="cps")

        def load_block(b, blk):
            encT = encT_pool.tile([P, n_hc, NQ, P], BF16, tag="encT")
            nc.gpsimd.dma_start(
                encT.rearrange("p c q s -> p (c q s)"), enc_t[b, blk, :, :]
            )
            nat = nat_pool.tile([P, NQ, H2], BF16, tag="nat")
            nc.gpsimd.dma_start(
                nat.rearrange("p q h -> p (q h)"), enc_n[b, blk, :, :]
            )
            return encT, nat

        # ---- software-pipelined main loop --------------------------------
        seq = [
            (b, blk)
            for _ in range(reps)
            for b in range(bloc)
            for blk in range(n_blk)
        ]
        pre_depth = 3
        prefetched = {}
        for i in range(min(pre_depth, len(seq))):
            prefetched[i] = load_block(*seq[i])

        batch_state = {}  # keyed by idx // n_blk

        def get_bstate(bidx, b):
            st = batch_state.get(bidx)
            if st is None:
                mt = row_pool.tile([P, NC16], F32, tag="mt")
                nc.sync.dma_start(mt.rearrange("p j -> p (j)"), mcol[b, :, :])
                ewgt = out_pool.tile([P, NC16], F32, tag="ewgt")
                st = batch_state[bidx] = {"mt": mt, "ewgt": ewgt}
            return st

        def issue_batch_tail1(b, st):
            """esum + unnormalized staging; cheap PE part right after the
            last context matmuls."""
            # esum replicated on every partition: ones[128,128]^T @ ewgt
            # puts each column's total in every row; reduce over the 16
            # columns then reciprocal -> inv on all partitions.
            esr = sc_ps.tile([P, SB], F32, tag="scps")  # reuse scps ring slot
            nc.tensor.matmul(
                esr[:, 0:NC16], ones_f32[:], st["ewgt"][:], start=True, stop=True
            )
            esum_rep = row_pool.tile([P, 1], F32, tag="esum_rep")
            nc.vector.reduce_sum(
                esum_rep[:], esr[:, 0:NC16], axis=mybir.AxisListType.X
            )
            inv_rep = row_pool.tile([P, 1], F32, tag="inv_rep")
            nc.vector.reciprocal(inv_rep[:], esum_rep[:])
            # stage the 4 context partial rows into the zeroed bf16 tile
            # (partition-aligned copies on the otherwise-idle DVE)
            for g in range(NQ):
                nc.vector.tensor_copy(
                    c4[32 * g : 32 * g + 1, :], cps[32 * g : 32 * g + 1, :]
                )
            wgt_cols = out_pool.tile([P, NC16], F32, tag="wgt_cols")
            nc.scalar.activation(
                wgt_cols[:], st["ewgt"][:], AF.Copy, scale=inv_rep[:]
            )
            return esr, inv_rep, wgt_cols

        def issue_batch_tail2(b, esr, inv_rep, wgt_cols):
            """PE-dependent tail: issued a few microseconds later (inside
            the next block's energy stream) so nothing here stalls the PE."""
            # sum the 4 partial rows: sel . c4 -> cps row 0, then scale
            for hh in range(n_hh):
                nc.tensor.matmul(
                    cps[0:1, hh * 512 : (hh + 1) * 512],
                    sel_bf[:],
                    c4[:, hh * 512 : (hh + 1) * 512],
                    start=True,
                    stop=True,
                    skip_group_check=True,
                )
            ctx_sb = out_pool.tile([1, H2], F32, tag="ctx_sb")
            for hh in range(n_hh):
                nc.scalar.activation(
                    ctx_sb[0:1, hh * 512 : (hh + 1) * 512],
                    cps[0:1, hh * 512 : (hh + 1) * 512],
                    AF.Copy,
                    scale=inv_rep[0:1, :],
                )
            nc.sync.dma_start(ctx_o[b : b + 1, :], ctx_sb[:])
            # transpose wgt columns -> rows on the PE (16-descriptor DMA
            # instead of a 2048 x 4B scatter)
            tps = energy_ps.tile([P, SB], F32, tag="eps")  # borrow eps slot
            nc.tensor.transpose(tps[0:NC16, 0:P], wgt_cols[:], ident[:])
            wgt_row = out_pool.tile([NC16, P], F32, tag="wgt_row")
            nc.vector.tensor_copy(wgt_row[:], tps[0:NC16, 0:P])
            nc.sync.dma_start(
                wgt_o[b : b + 1, :].rearrange("o (c p) -> (o c) p", p=P),
                wgt_row[:],
            )

        # Pending cross-block work (closures issued inside later blocks'
        # PE streams to keep the PE busy across the tanh->exp latency).
        pending_sT = []  # previous block's sT(ca3) matmuls, one per q
        pending_exp = []  # previous block's exp ACT ops
        pending_ctx = []  # previous block's context matmuls + batch tail1
        pending_tail2 = []  # batch tail part 2 (deferred past cross-engine chain)

        for idx, (b, blk) in enumerate(seq):
            st = get_bstate(idx // n_blk, b)
            encT, nat = prefetched.pop(idx)
            if idx + pre_depth < len(seq):
                prefetched[idx + pre_depth] = load_block(*seq[idx + pre_depth])
            if mode == "dma":
                continue

            scps = sc_ps.tile([P, SB], F32, tag="scps")
            et_l = []

            # scoresT psum: sT(ca0, q0) carries start=True (zeroes the
            # whole bank; later writes accumulate onto pending-zero).  The
            # sT fillers are issued AFTER the previous block's exp so the
            # recycled psum slot's WAR ordering is correct.
            def issue_sT_one(ca, q, scps=scps, et_l=et_l, last=False):
                nc.tensor.matmul(
                    scps[:, q : q + 1],
                    et_l[ca][:, q * P : (q + 1) * P],
                    vsb[:, ca : ca + 1],
                    start=(ca == 0 and q == 0),
                    stop=last,
                    skip_group_check=True,
                )

            for ca in range(n_ac):
                # fillers interleaved into this ca's energy stream
                if ca == 0:
                    # prev block's sT(ca3) + exp
                    fillers = pending_sT + pending_exp
                    pending_sT, pending_exp = [], []
                    fill_at = dict(zip((3, 5, 7, 9, 11), range(5)))
                else:
                    prev = ca - 1
                    fillers = [
                        (lambda q=q, prev=prev: issue_sT_one(prev, q))
                        for q in range(NQ)
                    ]
                    fill_at = {8: 0, 10: 1, 12: 2, 14: 3}

                if ca == 2:
                    for f in pending_tail2:
                        f()
                    pending_tail2 = []

                eps = energy_ps.tile([P, SB], F32, tag="eps")
                for c in range(n_hc):
                    nc.tensor.matmul(
                        eps[:],
                        whsb[:, c, ca * P : (ca + 1) * P],
                        encT[:, c, :, :],
                        start=(c == 0),
                        stop=(c == n_hc - 1),
                    )
                    fi = fill_at.get(c)
                    if fi is not None and fi < len(fillers):
                        fillers[fi]()
                for f in fillers[len(fill_at) :]:
                    f()
                et = et_pool.tile([P, SB], BF16, tag="et")
                et_l.append(et)
                nc.scalar.activation(
                    et[:], eps[:], AF.Tanh, bias=dpt[:, ca, b : b + 1]
                )

            # previous block's context matmuls (+ batch tail) fill the PE
            # while this block's tanh(ca3)->sT(ca3)->exp chain completes.
            for f in pending_ctx:
                f()
            pending_ctx = []

            # this block's trailing work, deferred into the next block
            pending_sT = [
                (
                    lambda q=q, scps=scps, et_l=et_l: issue_sT_one(
                        n_ac - 1, q, scps, et_l, last=(q == NQ - 1)
                    )
                )
                for q in range(NQ)
            ]

            ecol = ec_pool.tile([P, NQ], BF16, tag="ecol")

            def issue_exp(scps=scps, ecol=ecol, st=st, blk=blk):
                # add the log-mask columns (0 / -30000) on the idle DVE,
                # then exp; masked positions exp to exactly 0.
                scpm = ec_pool.tile([P, NQ], F32, tag="scpm")
                nc.vector.tensor_add(
                    scpm[:], scps[:, 0:NQ], st["mt"][:, blk * NQ : (blk + 1) * NQ]
                )
                nc.scalar.activation(ecol[:], scpm[:], AF.Exp)
                nc.scalar.activation(
                    st["ewgt"][:, blk * NQ : (blk + 1) * NQ],
                    scpm[:],
                    AF.Exp,
                )

            pending_exp = [issue_exp]

            def issue_ctx(
                b=b, blk=blk, ecol=ecol, nat=nat, st=st, bidx=idx // n_blk
            ):
                if mode != "noctx":
                    for q in range(NQ):
                        row = 32 * q
                        for hh in range(n_hh):
                            nc.tensor.matmul(
                                cps[row : row + 1, hh * 512 : (hh + 1) * 512],
                                ecol[:, q : q + 1],
                                nat[:, q, hh * 512 : (hh + 1) * 512],
                                start=(blk == 0),
                                stop=(blk == n_blk - 1),
                                tile_position=(0, row),
                                skip_group_check=True,
                            )
                if blk == n_blk - 1:
                    tail_args = issue_batch_tail1(b, st)
                    pending_tail2.append(
                        lambda b=b, ta=tail_args: issue_batch_tail2(b, *ta)
                    )
                    del batch_state[bidx]

            pending_ctx = [issue_ctx]

        # drain the pipeline tail
        for f in pending_sT + pending_exp + pending_ctx:
            f()
        for f in pending_tail2:
            f()

    return _spill_excess_waits(nc)


class _Runner:
    """Compile once, execute many times with device-resident inputs."""

    def __init__(self, bloc, s_len, n_cores=N_CORES):
        import jax
        from jax.experimental.shard_map import shard_map
        from jax.sharding import Mesh, PartitionSpec

        from concourse import bass2jax

        bass2jax.install_neuronx_cc_hook()
        self.n_cores = n_cores
        self.bloc = bloc
        nc = build_bass(bloc, s_len)
        in_names, out_names, out_avals = [], [], []
        for alloc in nc.m.functions[0].allocations:
            if not isinstance(alloc, mybir.MemoryLocationSet):
                continue
            name = alloc.memorylocations[0].name
            if alloc.kind == "ExternalInput":
                in_names.append(name)
            elif alloc.kind == "ExternalOutput":
                out_names.append(name)
                out_avals.append(
                    jax.core.ShapedArray(
                        tuple(alloc.tensor_shape), mybir.dt.np(alloc.dtype)
                    )
                )
        partition_name = (
            nc.partition_id_tensor.name if nc.partition_id_tensor else None
        )
        if partition_name is not None:
            in_names = [n for n in in_names if n != partition_name]
        self.in_names = in_names
        self.out_names = out_names
        self.out_avals = out_avals
        n_params = len(in_names)
        n_outs = len(out_names)
        all_in_names = tuple(in_names) + tuple(out_names)
        if partition_name is not None:
            all_in_names = all_in_names + (partition_name,)

        def _body(*args):
            operands = list(args)
            if partition_name is not None:
                operands.append(bass2jax.partition_id_tensor())
            outs = bass2jax._bass_exec_p.bind(
                *operands,
                out_avals=tuple(out_avals),
                in_names=all_in_names,
                out_names=tuple(out_names),
                lowering_input_output_aliases=(),
                sim_require_finite=True,
                sim_require_nnan=True,
                nc=nc,
            )
            return tuple(outs)

        devices = jax.devices()[:n_cores]
        self.mesh = Mesh(np.asarray(devices), ("core",))
        in_specs = (PartitionSpec("core"),) * (n_params + n_outs)
        out_specs = (PartitionSpec("core"),) * n_outs
        self.sharded = jax.jit(
            shard_map(
                _body,
                mesh=self.mesh,
                in_specs=in_specs,
                out_specs=out_specs,
                check_rep=False,
            ),
            donate_argnums=tuple(range(n_params, n_params + n_outs)),
            keep_unused=True,
        )
        self._jax = jax

    def put_inputs(self, per_core_maps):
        """per_core_maps: list of dicts name->np array (per-core shapes).
        Returns device arrays (concatenated on axis 0)."""
        import jax
        from jax.sharding import NamedSharding, PartitionSpec

        sh = NamedSharding(self.mesh, PartitionSpec("core"))
        arrs = []
        for name in self.in_names:
            cat = np.concatenate(
                [np.asarray(m[name]) for m in per_core_maps], axis=0
            )
            arrs.append(jax.device_put(cat, sh))
        jax.block_until_ready(arrs)
        return arrs

    def _zero_outs(self):
        return [
            np.zeros((self.n_cores * a.shape[0], *a.shape[1:]), a.dtype)
            for a in self.out_avals
        ]

    def run(self, dev_inputs):
        outs = self.sharded(*dev_inputs, *self._zero_outs())
        self._jax.block_until_ready(outs)
        return outs

    def run_np(self, dev_inputs):
        outs = self.run(dev_inputs)
        return {n: np.asarray(o) for n, o in zip(self.out_names, outs)}


_RUNNER_CACHE = {}


def _get_runner(bloc, s_len, n_cores=N_CORES):
    key = (bloc, s_len, n_cores)
    if key not in _RUNNER_CACHE:
        _RUNNER_CACHE[key] = _Runner(bloc, s_len, n_cores)
    return _RUNNER_CACHE[key]


def make_in_maps(decoder_hidden, encoder_outputs, mask, W_s, W_h, v, n_cores=N_CORES):
    import ml_dtypes

    bf16 = ml_dtypes.bfloat16
    b_full, s_len = mask.shape
    bloc = b_full // n_cores
    n_blk = s_len // 512
    h2 = encoder_outputs.shape[2]

    enc_bf = np.asarray(encoder_outputs, dtype=np.float32).astype(bf16)
    # enc_t[b, blk, p, c, q, s] = enc[b, blk*512+q*128+s, c*128+p]
    # (c before q so the energy matmul's moving operand is a contiguous
    # 512-element run per partition)
    e6 = enc_bf.reshape(b_full, n_blk, 4, 128, h2 // 128, 128)
    enc_t = np.ascontiguousarray(e6.transpose(0, 1, 5, 4, 2, 3)).reshape(
        b_full, n_blk, 128, -1
    )
    # enc_n[b, blk, p, q, h] = enc[b, blk*512+q*128+p, h]
    enc_n = np.ascontiguousarray(
        enc_bf.reshape(b_full, n_blk, 4, 128, h2).transpose(0, 1, 3, 2, 4)
    ).reshape(b_full, n_blk, 128, -1)

    dec_np = np.asarray(decoder_hidden, dtype=np.float32)
    ws_np = np.asarray(W_s, dtype=np.float32)
    dp = dec_np @ ws_np  # (B, A) f32
    a_dim = dp.shape[1]
    # dpt[p, ca, b] = dp[b, ca*128+p]
    dpt_full = np.ascontiguousarray(
        dp.reshape(b_full, a_dim // 128, 128).transpose(2, 1, 0)
    )

    wh_np = np.asarray(W_h, dtype=np.float32).astype(bf16)
    # whsb[p, c, a] = W_h[c*128+p, a]
    wh_t = np.ascontiguousarray(
        wh_np.reshape(h2 // 128, 128, a_dim).transpose(1, 0, 2)
    ).reshape(128, -1)
    v_np = np.asarray(v, dtype=np.float32).astype(bf16)
    v_t = np.ascontiguousarray(v_np.reshape(a_dim // 128, 128).T)

    mask_np = np.asarray(mask)
    # mcol[b, p, j] = log-mask of s = j*128 + p
    mcol = np.ascontiguousarray(
        np.where(mask_np, np.float32(0.0), np.float32(MASK_NEG))
        .reshape(b_full, s_len // 128, 128)
        .transpose(0, 2, 1)
    )
    sel_np = np.zeros((128, 1), np.float32)
    sel_np[::32] = 1.0
    sel_np = sel_np.astype(bf16)

    in_maps = []
    for i in range(n_cores):
        sl = slice(i * bloc, (i + 1) * bloc)
        in_maps.append(
            {
                "enc_t": enc_t[sl],
                "enc_n": enc_n[sl],
                "dpt": dpt_full[:, :, sl],
                "w_h": wh_t,
                "v_in": v_t,
                "mcol": mcol[sl],
                "csel": sel_np,
            }
        )
    return in_maps


def run_sharded(decoder_hidden, encoder_outputs, mask, W_s, W_h, v, n_cores=N_CORES):
    b_full, s_len = np.asarray(mask).shape
    bloc = b_full // n_cores
    runner = _get_runner(bloc, s_len, n_cores)
    in_maps = make_in_maps(
        decoder_hidden, encoder_outputs, mask, W_s, W_h, v, n_cores
    )
    dev_in = runner.put_inputs(in_maps)
    outs = runner.run_np(dev_in)
    ctx = outs["ctx_o"].reshape(b_full, H2)
    wgt = outs["wgt_o"].reshape(b_full, s_len)
    return ctx, wgt


def kernel(decoder_hidden, encoder_outputs, mask, W_s, W_h, v):
    decoder_hidden = np.asarray(decoder_hidden)
    encoder_outputs = np.asarray(encoder_outputs)
    mask = np.asarray(mask)
    W_s = np.asarray(W_s)
    W_h = np.asarray(W_h)
    v = np.asarray(v)
    ctx, wgt = run_sharded(decoder_hidden, encoder_outputs, mask, W_s, W_h, v)
    return ctx, wgt


# revision 39
# speedup vs baseline: 1.0050x; 1.0050x over previous
"""Bahdanau attention Trainium2 kernel (v2).

score(s, h_i) = v . tanh(W_s s + W_h h_i);  softmax over S;  context = w @ enc.

Strategy (per NeuronCore, data-parallel over batch, 8 batches/core):
  - Host pre-marshals enc into TWO bf16 layouts so the device does no
    casts and no SBUF transposes (which were ~230us of DMA + ~160us of
    DVE in v1):
      enc_t[b, blk, p, c, q, s]  (h = c*128+p, s_glob = blk*512+q*128+s)
        -> energy-GEMM moving tiles [128h, (c,q,s)], contiguous 512-run
           per partition per h-chunk
      enc_n[b, blk, p, q, h]     (s_glob = blk*512+q*128+p)
        -> context-GEMM moving tiles [128s, (q,h)]
    Total HBM traffic is unchanged (two bf16 copies = one f32 copy);
    everything rides the gpsimd SWDGE ring (sprays all 16 DMA engines).
  - dec_proj = dec @ W_s is computed on host (0.01% of FLOPs) and fed
    as the transposed ACT bias dpt[a_p, ca, b].
  - energy runs transposed per 512-s block: psum [a=128, s=512], W_h
    chunks (bf16) stationary; tanh folds dpt in as the ACT bias.
  - scoresT is produced directly as columns: per q, scT[s=128, 1] =
    sum_ca et[ca][:, q]^T v[ca] via N=1 matmuls whose 128-col LS hides
    under the 512-cycle energy matmuls they are interleaved with.
  - the log-mask (0 / -30000) columns are added on the idle DVE before
    exp, so masked positions exp to exactly 0; exp on ACT -> e columns
    (bf16 for the context lhsT, f32 for the weight output); no
    max-subtraction (|scores| <~ 20, f32 exp safe).
  - context accumulates into psum rows 0/32/64/96 via 4 PE column
    groups (their N=512 streams overlap on HW, measured ~2.8x); block-k
    context matmuls are issued inside block k+1's energy stream so the
    PE never waits on the tanh->exp chain.
  - batch tail: colsum-replicating f32 matmul (ones^T @ ewgt) -> DVE
    reduce+reciprocal -> ACT scales; the 4 context partial rows are
    summed by a sel matmul from a zeroed bf16 staging tile; the weight
    row is PE-transposed so its HBM write is 16 contiguous descriptors.

Measured dead ends (do not retry without new hardware/toolchain data):
  - fp8e4 DoubleRow matmul: ~1250 ns/MM on HW (vs 164 ns bf16 stream)
    despite the cost model's 0.5 cyc/row claim — walrus lowering is
    pathological.  fp8 numerics alone were also over budget (1.5-2.5e-2
    vs the 2e-2 gate for single/split/3-gemm variants).
  - DoublePixel / DoubleColumn perf modes: enum-only, no lowering.
  - W-stationary reuse across block pairs (would elide ~half the
    energy LS, ~70us): needs 4 psum banks for the eps pair + pipeline,
    but cps(4) + scps(2) leave only 2 of 8; 3-bank variants stall on
    tanh ~0.5us per ca-transition, eating the gain.
  - prefetch depth 4: SBUF overflow (outp pool charges full width per
    partition for single-partition tiles; ~4KB/partition short).
  - sim's TimelineSim tracks HW within ~2% in quiet windows; it does
    NOT model LS time (Ldweights engine cost = 0) nor tile_position
    col-group overlap (serializes them), so validate those on HW.
HW microbench rates (quiet window): bf16 [128x512xK=128] same-
stationary stream 140-164 ns/MM; fresh-stationary chains ~210 ns/MM;
M=1 N=512 serial ~310 ns/MM, ~110 ns/MM effective in 4 col groups.
"""

import sys
from contextlib import ExitStack

sys.path.insert(0, "/opt/trn_rl_repo")

import numpy as np

import concourse.bass as bass
import concourse.tile as tile
from concourse import mybir
from concourse.masks import make_identity

# ---- walrus workaround: tail drain accepts only 1 sync wait ----------------
from concourse.vector_clock import ScopedClock, VectorClock


def _patched_drain_and_barrier(self, tick_clock, wait_clock):
    gc = tick_clock.global_clock
    procs = [(i, gc[i]) for i in range(len(gc)) if gc[i] > 0]
    for p, t in procs:
        vc = VectorClock()
        vc.require_at_least(p, t)
        nop = self.nc.sync.nop(nofuse=True, hint="tail_wait_split")
        wait_clock.add_sem_waits(nop.ins, ScopedClock({None: vc}))
    self.nc.sync.drain()
    self.nc.all_engine_barrier()
    assert self.sems is not None
    popped = self.nc._tile_sem_poison_stack.pop()
    assert popped is self._sem_poison
    self.nc.clear_and_free_semaphores(list(self.sems.allocated().values()))
    self.nc.all_engine_barrier()


tile.TileContext._drain_and_barrier = _patched_drain_and_barrier


def _spill_excess_waits(nc):
    """This walrus build accepts at most 1 sync wait per instruction (2 for
    EventSemaphore).  Tile's wait assignment can attach several.  Move the
    excess onto same-engine NOPs inserted immediately before the
    instruction — NX sequencers process instructions in order, so the NOP
    stalls the engine exactly like an on-instruction wait would."""
    import bass_rust

    nop_id = [0]
    for fn in nc.m.functions:
        for blk in fn.blocks:
            new_insts = []
            changed = False
            for inst in blk.instructions:
                si = inst.sync_info
                cap = 2 if type(inst).__name__ == "InstEventSemaphore" else 1
                if si is not None and len(si.on_wait) > cap:
                    waits = list(si.on_wait)
                    keep, spill = waits[-cap:], waits[:-cap]
                    for w in spill:
                        nop = mybir.InstNoOp(
                            name=f"I-waitspill-{nop_id[0]}", ins=[], outs=[]
                        )
                        nop_id[0] += 1
                        nop.engine = inst.engine
                        nop.sync_info = bass_rust.SyncInfo(
                            on_wait=[w], on_update=[]
                        )
                        nc.register_instruction(nop, overwrite=True)
                        new_insts.append(nop)
                    inst.sync_info = bass_rust.SyncInfo(
                        on_wait=keep, on_update=list(si.on_update)
                    )
                    changed = True
                new_insts.append(inst)
            if changed:
                blk.instructions = new_insts
    return nc


# ---------------------------------------------------------------------------

N_CORES = 8
B, S, H, A = 64, 2048, 1024, 512
H2 = 2 * H
F32 = mybir.dt.float32
BF16 = mybir.dt.bfloat16
AF = mybir.ActivationFunctionType
MASK_NEG = -30000.0  # exp(score + MASK_NEG) == 0.0 in f32 for |score| <~ 1e3


def build_bass(bloc, s_len, reps=1, mode="full"):
    """One-core program processing bloc batch rows of length s_len.

    reps>1 re-emits the whole compute loop (idempotent writes) so the
    per-iteration device time can be measured as a wall-clock slope."""
    P = 128
    SB = 512  # s-block
    n_blk = s_len // SB
    n_hc = H2 // P  # 16 h-chunks in the projection contraction
    n_ac = A // P  # 4 a-chunks
    n_hh = H2 // 512  # 4 context output slices
    NQ = SB // P  # 4 s-subtiles per block
    NC16 = n_blk * NQ  # 16 e-columns per batch

    nc = bass.Bass("TRN2", target_bir_lowering=False, debug=False)
    enc_t = nc.dram_tensor(
        "enc_t", [bloc, n_blk, P, NQ * n_hc * P], BF16, kind="ExternalInput"
    ).ap()
    enc_n = nc.dram_tensor(
        "enc_n", [bloc, n_blk, P, NQ * H2], BF16, kind="ExternalInput"
    ).ap()
    dpt_in = nc.dram_tensor("dpt", [P, n_ac, bloc], F32, kind="ExternalInput").ap()
    w_h = nc.dram_tensor("w_h", [P, n_hc * A], BF16, kind="ExternalInput").ap()
    v_in = nc.dram_tensor("v_in", [P, n_ac], BF16, kind="ExternalInput").ap()
    mcol = nc.dram_tensor("mcol", [bloc, P, NC16], F32, kind="ExternalInput").ap()
    csel = nc.dram_tensor("csel", [P, 1], BF16, kind="ExternalInput").ap()
    ctx_o = nc.dram_tensor("ctx_o", [bloc, H2], F32, kind="ExternalOutput").ap()
    wgt_o = nc.dram_tensor("wgt_o", [bloc, s_len], F32, kind="ExternalOutput").ap()

    with tile.TileContext(nc) as tc, ExitStack() as ctx:
        consts = ctx.enter_context(tc.tile_pool(name="consts", bufs=1))

        whsb = consts.tile([P, n_hc, A], BF16)
        nc.sync.dma_start(whsb.rearrange("p c a -> p (c a)"), w_h[:, :])
        vsb = consts.tile([P, n_ac], BF16)
        nc.sync.dma_start(vsb[:], v_in[:, :])
        dpt = consts.tile([P, n_ac, bloc], F32)
        nc.sync.dma_start(dpt.rearrange("p c b -> p (c b)"), dpt_in[:, :, :])
        sel_bf = consts.tile([P, 1], BF16)
        nc.sync.dma_start(sel_bf[:], csel[:])
        ones_f32 = consts.tile([P, P], F32)
        nc.gpsimd.memset(ones_f32[:], 1.0)
        ident = consts.tile([P, P], F32)
        make_identity(nc, ident[:])
        # persistent bf16 staging for the 4 context partial rows; zeroed
        # once so the sel matmul's unused partitions contribute exact 0.
        c4 = consts.tile([P, H2], BF16)
        nc.gpsimd.memset(c4[:], 0.0)

        encT_pool = ctx.enter_context(tc.tile_pool(name="encT", bufs=4))
        nat_pool = ctx.enter_context(tc.tile_pool(name="nat", bufs=5))
        et_pool = ctx.enter_context(tc.tile_pool(name="et", bufs=6))
        ec_pool = ctx.enter_context(tc.tile_pool(name="ec", bufs=3))
        row_pool = ctx.enter_context(tc.tile_pool(name="row", bufs=2))
        out_pool = ctx.enter_context(tc.tile_pool(name="outp", bufs=2))
        energy_ps = ctx.enter_context(
            tc.tile_pool(name="energy_ps", bufs=2, space="PSUM")
        )
        sc_ps = ctx.enter_context(tc.tile_pool(name="sc_ps", bufs=2, space="PSUM"))
        ctx_ps = ctx.enter_context(tc.tile_pool(name="ctx_ps", bufs=1, space="PSUM"))

        # context psum lives across the whole kernel; the 4 q-groups write
        # partial rows 0/32/64/96 in separate PE column groups (their N=512
        # streams overlap on HW); each batch's first matmul per group
        # re-initializes its row via start=True.
        cps = ctx_ps.tile([P, H2], F32, tag="cps")

        def load_block(b, blk):
            encT = encT_pool.tile([P, n_hc, NQ, P], BF16, tag="encT")
            nc.gpsimd.dma_start(
                encT.rearrange("p c q s -> p (c q s)"), enc_t[b, blk, :, :]
            )
            nat = nat_pool.tile([P, NQ, H2], BF16, tag="nat")
            nc.gpsimd.dma_start(
                nat.rearrange("p q h -> p (q h)"), enc_n[b, blk, :, :]
            )
            return encT, nat

        # ---- software-pipelined main loop --------------------------------
        seq = [
            (b, blk)
            for _ in range(reps)
            for b in range(bloc)
            for blk in range(n_blk)
        ]
        pre_depth = 3
        prefetched = {}
        for i in range(min(pre_depth, len(seq))):
            prefetched[i] = load_block(*seq[i])

        batch_state = {}  # keyed by idx // n_blk

        def get_bstate(bidx, b):
            st = batch_state.get(bidx)
            if st is None:
                mt = row_pool.tile([P, NC16], F32, tag="mt")
                nc.sync.dma_start(mt.rearrange("p j -> p (j)"), mcol[b, :, :])
                ewgt = out_pool.tile([P, NC16], F32, tag="ewgt")
                st = batch_state[bidx] = {"mt": mt, "ewgt": ewgt}
            return st

        def issue_batch_tail1(b, st):
            """esum + unnormalized staging; cheap PE part right after the
            last context matmuls."""
            # esum replicated on every partition: ones[128,128]^T @ ewgt
            # puts each column's total in every row; reduce over the 16
            # columns then reciprocal -> inv on all partitions.
            esr = sc_ps.tile([P, SB], F32, tag="scps")  # reuse scps ring slot
            nc.tensor.matmul(
                esr[:, 0:NC16], ones_f32[:], st["ewgt"][:], start=True, stop=True
            )
            esum_rep = row_pool.tile([P, 1], F32, tag="esum_rep")
            nc.vector.reduce_sum(
                esum_rep[:], esr[:, 0:NC16], axis=mybir.AxisListType.X
            )
            inv_rep = row_pool.tile([P, 1], F32, tag="inv_rep")
            nc.vector.reciprocal(inv_rep[:], esum_rep[:])
            # stage the 4 context partial rows into the zeroed bf16 tile
            # (partition-aligned copies on the otherwise-idle DVE)
            for g in range(NQ):
                nc.vector.tensor_copy(
                    c4[32 * g : 32 * g + 1, :], cps[32 * g : 32 * g + 1, :]
                )
            wgt_cols = out_pool.tile([P, NC16], F32, tag="wgt_cols")
            nc.scalar.activation(
                wgt_cols[:], st["ewgt"][:], AF.Copy, scale=inv_rep[:]
            )
            return esr, inv_rep, wgt_cols

        def issue_batch_tail2(b, esr, inv_rep, wgt_cols):
            """PE-dependent tail: issued a few microseconds later (inside
            the next block's energy stream) so nothing here stalls the PE."""
            # sum the 4 partial rows: sel . c4 -> cps row 0, then scale
            for hh in range(n_hh):
                nc.tensor.matmul(
                    cps[0:1, hh * 512 : (hh + 1) * 512],
                    sel_bf[:],
                    c4[:, hh * 512 : (hh + 1) * 512],
                    start=True,
                    stop=True,
                    skip_group_check=True,
                )
            ctx_sb = out_pool.tile([1, H2], F32, tag="ctx_sb")
            for hh in range(n_hh):
                nc.scalar.activation(
                    ctx_sb[0:1, hh * 512 : (hh + 1) * 512],
                    cps[0:1, hh * 512 : (hh + 1) * 512],
                    AF.Copy,
                    scale=inv_rep[0:1, :],
                )
            nc.sync.dma_start(ctx_o[b : b + 1, :], ctx_sb[:])
            # transpose wgt columns -> rows on the PE (16-descriptor DMA
            # instead of a 2048 x 4B scatter)
            tps = energy_ps.tile([P, SB], F32, tag="eps")  # borrow eps slot
            nc.tensor.transpose(tps[0:NC16, 0:P], wgt_cols[:], ident[:])
            wgt_row = out_pool.tile([NC16, P], F32, tag="wgt_row")
            nc.vector.tensor_copy(wgt_row[:], tps[0:NC16, 0:P])
            nc.sync.dma_start(
                wgt_o[b : b + 1, :].rearrange("o (c p) -> (o c) p", p=P),
                wgt_row[:],
            )

        # Pending cross-block work (closures issued inside later blocks'
        # PE streams to keep the PE busy across the tanh->exp latency).
        pending_sT = []  # previous block's sT(ca3) matmuls, one per q
        pending_exp = []  # previous block's exp ACT ops
        pending_ctx = []  # previous block's context matmuls + batch tail1
        pending_tail2 = []  # batch tail part 2 (deferred past cross-engine chain)

        for idx, (b, blk) in enumerate(seq):
            st = get_bstate(idx // n_blk, b)
            encT, nat = prefetched.pop(idx)
            if idx + pre_depth < len(seq):
                prefetched[idx + pre_depth] = load_block(*seq[idx + pre_depth])
            if mode == "dma":
                continue

            scps = sc_ps.tile([P, SB], F32, tag="scps")
            et_l = []

            # scoresT psum: sT(ca0, q0) carries start=True (zeroes the
            # whole bank; later writes accumulate onto pending-zero).  The
            # sT fillers are issued AFTER the previous block's exp so the
            # recycled psum slot's WAR ordering is correct.
            def issue_sT_one(ca, q, scps=scps, et_l=et_l, last=False):
                nc.tensor.matmul(
                    scps[:, q : q + 1],
                    et_l[ca][:, q * P : (q + 1) * P],
                    vsb[:, ca : ca + 1],
                    start=(ca == 0 and q == 0),
                    stop=last,
                    skip_group_check=True,
                )

            for ca in range(n_ac):
                # fillers interleaved into this ca's energy stream
                if ca == 0:
                    # prev block's sT(ca3) + exp
                    fillers = pending_sT + pending_exp
                    pending_sT, pending_exp = [], []
                    fill_at = dict(zip((3, 5, 7, 9, 11), range(5)))
                else:
                    prev = ca - 1
                    fillers = [
                        (lambda q=q, prev=prev: issue_sT_one(prev, q))
                        for q in range(NQ)
                    ]
                    fill_at = {8: 0, 10: 1, 12: 2, 14: 3}

                if ca == 2:
                    for f in pending_tail2:
                        f()
                    pending_tail2 = []

                eps = energy_ps.tile([P, SB], F32, tag="eps")
                for c in range(n_hc):
                    nc.tensor.matmul(
                        eps[:],
                        whsb[:, c, ca * P : (ca + 1) * P],
                        encT[:, c, :, :],
                        start=(c == 0),
                        stop=(c == n_hc - 1),
                    )
                    fi = fill_at.get(c)
                    if fi is not None and fi < len(fillers):
                        fillers[fi]()
                for f in fillers[len(fill_at) :]:
                    f()
                et = et_pool.tile([P, SB], BF16, tag="et")
                et_l.append(et)
                nc.scalar.activation(
                    et[:], eps[:], AF.Tanh, bias=dpt[:, ca, b : b + 1]
                )

            # previous block's context matmuls (+ batch tail) fill the PE
            # while this block's tanh(ca3)->sT(ca3)->exp chain completes.
            for f in pending_ctx:
                f()
            pending_ctx = []

            # this block's trailing work, deferred into the next block
            pending_sT = [
                (
                    lambda q=q, scps=scps, et_l=et_l: issue_sT_one(
                        n_ac - 1, q, scps, et_l, last=(q == NQ - 1)
                    )
                )
                for q in range(NQ)
            ]

            ecol = ec_pool.tile([P, NQ], BF16, tag="ecol")

            def issue_exp(scps=scps, ecol=ecol, st=st, blk=blk):
                # add the log-mask columns (0 / -30000) on the idle DVE,
                # then exp; masked positions exp to exactly 0.
                scpm = ec_pool.tile([P, NQ], F32, tag="scpm")
                nc.vector.tensor_add(
                    scpm[:], scps[:, 0:NQ], st["mt"][:, blk * NQ : (blk + 1) * NQ]
                )
                nc.scalar.activation(ecol[:], scpm[:], AF.Exp)
                nc.scalar.activation(
                    st["ewgt"][:, blk * NQ : (blk + 1) * NQ],
                    scpm[:],
                    AF.Exp,
                )

            pending_exp = [issue_exp]

            def issue_ctx(
                b=b, blk=blk, ecol=ecol, nat=nat, st=st, bidx=idx // n_blk
            ):
                if mode != "noctx":
                    for q in range(NQ):
                        row = 32 * q
                        for hh in range(n_hh):
                            nc.tensor.matmul(
                                cps[row : row + 1, hh * 512 : (hh + 1) * 512],
                                ecol[:, q : q + 1],
                                nat[:, q, hh * 512 : (hh + 1) * 512],
                                start=(blk == 0),
                                stop=(blk == n_blk - 1),
                                tile_position=(0, row),
                                skip_group_check=True,
                            )
                if blk == n_blk - 1:
                    tail_args = issue_batch_tail1(b, st)
                    pending_tail2.append(
                        lambda b=b, ta=tail_args: issue_batch_tail2(b, *ta)
                    )
                    del batch_state[bidx]

            pending_ctx = [issue_ctx]

        # drain the pipeline tail
        for f in pending_sT + pending_exp + pending_ctx:
            f()
        for f in pending_tail2:
            f()

    return _spill_excess_waits(nc)


class _Runner:
    """Compile once, execute many times with device-resident inputs."""

    def __init__(self, bloc, s_len, n_cores=N_CORES):
        import jax
        from jax.experimental.shard_map import shard_map
        from jax.sharding import Mesh, PartitionSpec

        from concourse import bass2jax

        bass2jax.install_neuronx_cc_hook()
        self.n_cores = n_cores
        self.bloc = bloc
        nc = build_bass(bloc, s_len)
        in_names, out_names, out_avals = [], [], []
        for alloc in nc.m.functions[0].allocations:
            if not isinstance(alloc, mybir.MemoryLocationSet):
                continue
            name = alloc.memorylocations[0].name
            if alloc.kind == "ExternalInput":
                in_names.append(name)
            elif alloc.kind == "ExternalOutput":
                out_names.append(name)
                out_avals.append(
                    jax.core.ShapedArray(
                        tuple(alloc.tensor_shape), mybir.dt.np(alloc.dtype)
                    )
                )
        partition_name = (
            nc.partition_id_tensor.name if nc.partition_id_tensor else None
        )
        if partition_name is not None:
            in_names = [n for n in in_names if n != partition_name]
        self.in_names = in_names
        self.out_names = out_names
        self.out_avals = out_avals
        n_params = len(in_names)
        n_outs = len(out_names)
        all_in_names = tuple(in_names) + tuple(out_names)
        if partition_name is not None:
            all_in_names = all_in_names + (partition_name,)

        def _body(*args):
            operands = list(args)
            if partition_name is not None:
                operands.append(bass2jax.partition_id_tensor())
            outs = bass2jax._bass_exec_p.bind(
                *operands,
                out_avals=tuple(out_avals),
                in_names=all_in_names,
                out_names=tuple(out_names),
                lowering_input_output_aliases=(),
                sim_require_finite=True,
                sim_require_nnan=True,
                nc=nc,
            )
            return tuple(outs)

        devices = jax.devices()[:n_cores]
        self.mesh = Mesh(np.asarray(devices), ("core",))
        in_specs = (PartitionSpec("core"),) * (n_params + n_outs)
        out_specs = (PartitionSpec("core"),) * n_outs
        self.sharded = jax.jit(
            shard_map(
                _body,
                mesh=self.mesh,
                in_specs=in_specs,
                out_specs=out_specs,
                check_rep=False,
            ),
            donate_argnums=tuple(range(n_params, n_params + n_outs)),
            keep_unused=True,
        )
        self._jax = jax

    def put_inputs(self, per_core_maps):
        """per_core_maps: list of dicts name->np array (per-core shapes).
        Returns device arrays (concatenated on axis 0)."""
        import jax
        from jax.sharding import NamedSharding, PartitionSpec

        sh = NamedSharding(self.mesh, PartitionSpec("core"))
        arrs = []
        for name in self.in_names:
            cat = np.concatenate(
                [np.asarray(m[name]) for m in per_core_maps], axis=0
            )
            arrs.append(jax.device_put(cat, sh))
        jax.block_until_ready(arrs)
        return arrs

    def _zero_outs(self):
        return [
            np.zeros((self.n_cores * a.shape[0], *a.shape[1:]), a.dtype)
            for a in self.out_avals
        ]

    def run(self, dev_inputs):
        outs = self.sharded(*dev_inputs, *self._zero_outs())
        self._jax.block_until_ready(outs)
        return outs

    def run_np(self, dev_inputs):
        outs = self.run(dev_inputs)
        return {n: np.asarray(o) for n, o in zip(self.out_names, outs)}


_RUNNER_CACHE = {}


def _get_runner(bloc, s_len, n_cores=N_CORES):
    key = (bloc, s_len, n_cores)
    if key not in _RUNNER_CACHE:
        _RUNNER_CACHE[key] = _Runner(bloc, s_len, n_cores)
    return _RUNNER_CACHE[key]


def make_in_maps(decoder_hidden, encoder_outputs, mask, W_s, W_h, v, n_cores=N_CORES):
    import ml_dtypes

    bf16 = ml_dtypes.bfloat16
    b_full, s_len = mask.shape
    bloc = b_full // n_cores
    n_blk = s_len // 512
    h2 = encoder_outputs.shape[2]

    enc_bf = np.asarray(encoder_outputs, dtype=np.float32).astype(bf16)
    # enc_t[b, blk, p, c, q, s] = enc[b, blk*512+q*128+s, c*128+p]
    # (c before q so the energy matmul's moving operand is a contiguous
    # 512-element run per partition)
    e6 = enc_bf.reshape(b_full, n_blk, 4, 128, h2 // 128, 128)
    enc_t = np.ascontiguousarray(e6.transpose(0, 1, 5, 4, 2, 3)).reshape(
        b_full, n_blk, 128, -1
    )
    # enc_n[b, blk, p, q, h] = enc[b, blk*512+q*128+p, h]
    enc_n = np.ascontiguousarray(
        enc_bf.reshape(b_full, n_blk, 4, 128, h2).transpose(0, 1, 3, 2, 4)
    ).reshape(b_full, n_blk, 128, -1)

    dec_np = np.asarray(decoder_hidden, dtype=np.float32)
    ws_np = np.asarray(W_s, dtype=np.float32)
    dp = dec_np @ ws_np  # (B, A) f32
    a_dim = dp.shape[1]
    # dpt[p, ca, b] = dp[b, ca*128+p]
    dpt_full = np.ascontiguousarray(
        dp.reshape(b_full, a_dim // 128, 128).transpose(2, 1, 0)
    )

    wh_np = np.asarray(W_h, dtype=np.float32).astype(bf16)
    # whsb[p, c, a] = W_h[c*128+p, a]
    wh_t = np.ascontiguousarray(
        wh_np.reshape(h2 // 128, 128, a_dim).transpose(1, 0, 2)
    ).reshape(128, -1)
    v_np = np.asarray(v, dtype=np.float32).astype(bf16)
    v_t = np.ascontiguousarray(v_np.reshape(a_dim // 128, 128).T)

    mask_np = np.asarray(mask)
    # mcol[b, p, j] = log-mask of s = j*128 + p
    mcol = np.ascontiguousarray(
        np.where(mask_np, np.float32(0.0), np.float32(MASK_NEG))
        .reshape(b_full, s_len // 128, 128)
        .transpose(0, 2, 1)
    )
    sel_np = np.zeros((128, 1), np.float32)
    sel_np[::32] = 1.0
    sel_np = sel_np.astype(bf16)

    in_maps = []
    for i in range(n_cores):
        sl = slice(i * bloc, (i + 1) * bloc)
        in_maps.append(
            {
                "enc_t": enc_t[sl],
                "enc_n": enc_n[sl],
                "dpt": dpt_full[:, :, sl],
                "w_h": wh_t,
                "v_in": v_t,
                "mcol": mcol[sl],
                "csel": sel_np,
            }
        )
    return in_maps


def run_sharded(decoder_hidden, encoder_outputs, mask, W_s, W_h, v, n_cores=N_CORES):
    b_full, s_len = np.asarray(mask).shape
    bloc = b_full // n_cores
    runner = _get_runner(bloc, s_len, n_cores)
    in_maps = make_in_maps(
        decoder_hidden, encoder_outputs, mask, W_s, W_h, v, n_cores
    )
    dev_in = runner.put_inputs(in_maps)
    outs = runner.run_np(dev_in)
    ctx = outs["ctx_o"].reshape(b_full, H2)
    wgt = outs["wgt_o"].reshape(b_full, s_len)
    return ctx, wgt


def kernel(decoder_hidden, encoder_outputs, mask, W_s, W_h, v):
    decoder_hidden = np.asarray(decoder_hidden)
    encoder_outputs = np.asarray(encoder_outputs)
    mask = np.asarray(mask)
    W_s = np.asarray(W_s)
    W_h = np.asarray(W_h)
    v = np.asarray(v)
    ctx, wgt = run_sharded(decoder_hidden, encoder_outputs, mask, W_s, W_h, v)
    return ctx, wgt
